# revision 18
# baseline (speedup 1.0000x reference)
"""Trainium2 Bass kernel for AttentionDownsampler (nn_AttentionDownsampler_10264971837445).

Math (per batch b):
  patches[b, Y, X, p=(y,xi), c] = hr[b, c, 14Y+y, 14X+xi]
  logits[b, Y, X, p] = sum_c patches * w[c] + ab
  l2 = logits * mask[b, Y, X] * wkk[p] + bkk[p]
  attn = softmax_p(l2)
  out[b, c, Y, X] = sum_p patches[..., p, c] * attn[p]

Sharding: 8 cores = 4 batches x 2 halves of the H(=Y) axis; per-core shard is
8 patch-rows x 16 X x 196 px x 384 c, shipped as fp16 (halves HBM traffic;
rel-err budget is 2e-2, fp16 end-to-end lands ~1e-3).

Per-core kernel, processed in 4 groups of 2 patch-rows (rX = 32 patches):
  - DMA 3 c-chunk tiles [128, 2, 16, 196] fp16
  - PE scoring: 96 matmuls into one PSUM tile lg[32, 196]; the one-hot
    stationary [128, 32] for column rX is a shifted window of a padded
    [128, 255] tensor (col 127 = w chunk), so LDWEIGHTS is 32 cols and the
    stationary library stays tiny.
  - batched softmax over p on [32, 196]: DVE affine (x2 TT), reduce-max,
    ACT exp (bias=-max, fp16 out, accum sum), DVE reciprocal + scale -> fp16
  - PE broadcast: for each patch, matmul(one-hot-row [32,128], attn[32,196])
    replicates that patch's attn over all 128 partitions; outputs land in
    [128, 4, 256]-padded PSUM tiles (each [128,196] slice bank-aligned)
  - ACT evacuates PSUM->SBUF fp16 in quarter-row [128, 4, 196] ops
  - DVE pass B per (chunk, row): one big multiply [128, 3136] (fp16 2x mode)
    + one segmented tensor_reduce(axis=X) [128, 16, 196] -> [128, 16] fp32
    written straight into the output accumulator tile.
"""

import sys

for _p in ("/opt/trn_rl_repo", "/root/.axon_site/_ro/trn_rl_repo"):
    if _p not in sys.path:
        sys.path.append(_p)

import numpy as np

import concourse.bacc as bacc
import concourse.mybir as mybir
import concourse.tile as tile
from concourse.bass_utils import run_bass_kernel_spmd

K = 14          # patch size
C = 384         # channels
CCH = 128       # channel chunk (partitions)
NCH = C // CCH  # 3 chunks
NX = 16         # patches across W
P = K * K       # 196 pixels per patch
NROW = 8        # patch rows per core
NCORES = 8
GR = 2          # rows per group
NG = NROW // GR
GP = GR * NX    # patches per group (32)

FP32 = mybir.dt.float32
FP16 = mybir.dt.float16


def build_nc():
    nc = bacc.Bacc("TRN2", target_bir_lowering=False, debug=False,
                   num_devices=NCORES)

    hr = nc.dram_tensor("hr", [C, NROW, NX, P], FP16, kind="ExternalInput")
    # padded one-hot scorer weights: woh[c, k, 127] = w16[k*128+c], else 0
    woh = nc.dram_tensor("woh", [CCH, NCH, 255], FP16, kind="ExternalInput")
    # one-hot row selectors for the broadcast: bc[q, t, m] = (q == t)
    bc = nc.dram_tensor("bc", [GP, GP, CCH], FP16, kind="ExternalInput")
    # [GP, NG, P]: group index on the free axis so every DVE op reads
    # partitions 0..31 (DVE lanes are partition-hardwired)
    mwB = nc.dram_tensor("mwB", [GP, NG, P], FP32, kind="ExternalInput")
    bkkB = nc.dram_tensor("bkkB", [GP, NG, P], FP32, kind="ExternalInput")
    out = nc.dram_tensor("out", [C, NROW, NX], FP32, kind="ExternalOutput")

    with tile.TileContext(nc) as tc:
        _emit(tc, nc, hr, woh, bc, mwB, bkkB, out)
    nc.finalize()
    return nc


def _emit(tc, nc, hr, woh, bc, mwB, bkkB, out):
    import contextlib
    ctx = contextlib.ExitStack()
    with ctx:
        singles = ctx.enter_context(tc.tile_pool(name="singles", bufs=1))
        data_pool = ctx.enter_context(tc.tile_pool(name="data", bufs=9))
        small = ctx.enter_context(tc.tile_pool(name="small", bufs=8))
        attnb_pool = ctx.enter_context(tc.tile_pool(name="attnb", bufs=3))
        prod_pool = ctx.enter_context(tc.tile_pool(name="prod", bufs=3))
        prod2_pool = ctx.enter_context(tc.tile_pool(name="prod2", bufs=3))
        scratch_pool = ctx.enter_context(tc.tile_pool(name="scratch", bufs=1))
        psum_lg = ctx.enter_context(
            tc.tile_pool(name="psum_lg", bufs=3, space="PSUM"))
        psum_bc = ctx.enter_context(
            tc.tile_pool(name="psum_bc", bufs=2, space="PSUM"))

        # ---- constants (loaded once) ----
        woh_sb = singles.tile([CCH, NCH, 255], FP16)
        nc.sync.dma_start(out=woh_sb, in_=woh[:, :, :])
        bc_sb = singles.tile([GP, GP, CCH], FP16)
        nc.sync.dma_start(out=bc_sb, in_=bc[:, :, :])
        mwB_sb = singles.tile([GP, NG, P], FP32)
        nc.sync.dma_start(out=mwB_sb, in_=mwB[:, :, :])
        bkkB_sb = singles.tile([GP, NG, P], FP32)
        nc.sync.dma_start(out=bkkB_sb, in_=bkkB[:, :, :])

        osb = singles.tile([CCH, NCH, NROW, NX], FP32)
        act_scr = scratch_pool.tile([CCH, P // 2], FP32, tag="act_scr")

        dk_all = {}
        attn_all = {}

        def front_half(g):
            """DMA + scoring for group g (keeps PE fed ahead of the DVE)."""
            r0 = GR * g
            dk = []
            for k in range(NCH):
                t = data_pool.tile([CCH, GR, NX, P], FP16, tag="data")
                nc.sync.dma_start(
                    out=t, in_=hr[k * CCH:(k + 1) * CCH, r0:r0 + GR, :, :])
                dk.append(t)
            dk_all[g] = dk

            lg = psum_lg.tile([GP, P], FP32, tag="lg")
            for ri in range(GR):
                for X in range(NX):
                    col = NX * ri + X
                    for k in range(NCH):
                        nc.tensor.matmul(
                            lg[:, :],
                            woh_sb[:, k, 127 - col:127 - col + GP],
                            dk[k][:, ri, X, :],
                            start=(ri == 0 and X == 0 and k == 0),
                            stop=(ri == GR - 1 and X == NX - 1 and k == NCH - 1),
                        )
            return lg

        def softmax_group(g, lg):
            l2 = small.tile([GP, P], FP32, tag="l2")
            nc.vector.tensor_mul(l2, lg[:, :], mwB_sb[:, g, :])
            nc.vector.tensor_add(l2, l2, bkkB_sb[:, g, :])
            negmax = small.tile([GP, 1], FP32, tag="negmax")
            nc.vector.tensor_reduce(negmax, l2, axis=mybir.AxisListType.X,
                                    op=mybir.AluOpType.max, negate=True)
            ex16 = small.tile([GP, P], FP16, tag="ex16")
            esum = small.tile([GP, 1], FP32, tag="esum")
            nc.scalar.activation(ex16, l2, mybir.ActivationFunctionType.Exp,
                                 bias=negmax[:, 0:1], scale=1.0,
                                 accum_out=esum[:, 0:1])
            rcp = small.tile([GP, 1], FP32, tag="rcp")
            nc.vector.reciprocal(rcp, esum)
            attn16 = small.tile([GP, P], FP16, tag="attn16")
            nc.vector.tensor_scalar_mul(attn16, ex16, rcp[:, 0:1])
            return attn16

        def back_half(g, attn16):
            """broadcast + evac + multiply/fold/reduce for group g."""
            dk = dk_all[g]
            for ri in range(GR):
                r = GR * g + ri
                aB = attnb_pool.tile([CCH, NX, P], FP16, tag="aB")
                for q in range(4):          # quarter-rows of 4 patches
                    abp = psum_bc.tile([CCH, 4, 256], FP32, tag="abp")
                    for j in range(4):
                        t_loc = NX * ri + 4 * q + j
                        nc.tensor.matmul(
                            abp[:, j, 0:P],
                            bc_sb[:, t_loc, :],
                            attn16[:, :],
                            start=True, stop=True,
                        )
                    nc.scalar.activation(
                        aB[:, 4 * q:4 * q + 4, :], abp[:, :, 0:P],
                        mybir.ActivationFunctionType.Copy)

                for k in range(NCH):
                    prod = prod_pool.tile([CCH, NX, P], FP16, tag="prod")
                    if k == NCH - 1 and ri == GR - 1:
                        # probe: one multiply per group on the idle gpsimd
                        nc.gpsimd.tensor_mul(prod, dk[k][:, ri, :, :], aB)
                    else:
                        nc.vector.tensor_mul(prod, dk[k][:, ri, :, :], aB)
                    prod2 = prod2_pool.tile([CCH, NX, P // 2], FP16,
                                            tag="prod2")
                    nc.vector.tensor_add(prod2, prod[:, :, 0:P // 2],
                                         prod[:, :, P // 2:P])
                    nc.vector.tensor_reduce(
                        osb[:, k, r, :], prod2, axis=mybir.AxisListType.X,
                        op=mybir.AluOpType.add)

        # depth-2 software pipeline: PE scores two groups ahead so the DVE
        # never waits on scoring, and scoring never queues behind broadcasts
        lgs = {}
        for g in range(NG + 2):
            if g < NG:
                lgs[g] = front_half(g)
            if g >= 2:
                attn16 = softmax_group(g - 2, lgs.pop(g - 2))
                back_half(g - 2, attn16)

        for k in range(NCH):
            nc.sync.dma_start(out=out[k * CCH:(k + 1) * CCH, :, :],
                              in_=osb[:, k, :, :])


_NC_CACHE = {}


def _get_nc():
    if "nc" not in _NC_CACHE:
        _NC_CACHE["nc"] = build_nc()
    return _NC_CACHE["nc"]


def make_in_maps(hr_feats, guidance, attn_w, attn_b, w_kk, b_kk, dropout_mask):
    b = hr_feats.shape[0]
    w16 = np.asarray(attn_w, np.float32)[0].astype(np.float16)    # [384]
    ab = np.float32(np.asarray(attn_b, np.float32)[0])
    wkk_flat = np.asarray(w_kk, np.float32).reshape(-1)           # [196]
    bkk_flat = np.asarray(b_kk, np.float32).reshape(-1)
    mask = np.asarray(dropout_mask).astype(np.float32)[..., 0]    # [b, H, W]

    woh = np.zeros((CCH, NCH, 255), np.float16)
    woh[:, :, 127] = w16.reshape(NCH, CCH).T
    bc = np.zeros((GP, GP, CCH), np.float16)
    bc[np.arange(GP), np.arange(GP), :] = np.float16(1.0)

    in_maps = []
    for core in range(NCORES):
        bi, half = divmod(core, 2)
        bi = bi % b
        sl = np.asarray(hr_feats[bi, :, 112 * half:112 * half + K * NROW, :],
                        np.float32)
        hrg = sl.reshape(C, NROW, K, NX, K).transpose(0, 1, 3, 2, 4)
        hrg = np.ascontiguousarray(
            hrg.reshape(C, NROW, NX, P), np.float16)
        mask_flat = np.ascontiguousarray(
            mask[bi, NROW * half:NROW * half + NROW, :]).reshape(-1)  # [128]
        mwB = (mask_flat[:, None] * wkk_flat[None, :]).astype(np.float32)
        bkkB = (ab * mwB + bkk_flat[None, :]).astype(np.float32)
        # [128, 196] -> [GP, NG, P] with group index on the free axis
        mwB = np.ascontiguousarray(
            mwB.reshape(NG, GP, P).transpose(1, 0, 2))
        bkkB = np.ascontiguousarray(
            bkkB.reshape(NG, GP, P).transpose(1, 0, 2))
        in_maps.append({
            "hr": hrg, "woh": woh, "bc": bc, "mwB": mwB, "bkkB": bkkB,
        })
    return in_maps


def kernel(hr_feats, guidance, attn_w, attn_b, w_kk, b_kk, dropout_mask,
           trace=False):
    hr_feats = np.asarray(hr_feats, np.float32)
    b = hr_feats.shape[0]
    H = hr_feats.shape[2] // K
    nc = _get_nc()
    in_maps = make_in_maps(hr_feats, guidance, attn_w, attn_b, w_kk, b_kk,
                           dropout_mask)
    res = run_bass_kernel_spmd(nc, in_maps, core_ids=list(range(NCORES)),
                               trace=trace)
    full = np.empty((b, C, H, NX), np.float32)
    for core in range(NCORES):
        bi, half = divmod(core, 2)
        full[bi, :, NROW * half:NROW * half + NROW, :] = \
            res.results[core]["out"]
    if trace:
        return full, res
    return full


# revision 21
# speedup vs baseline: 1.0619x; 1.0619x over previous
"""Trainium2 Bass kernel for AttentionDownsampler (nn_AttentionDownsampler_10264971837445).

Math (per batch b):
  patches[b, Y, X, p=(y,xi), c] = hr[b, c, 14Y+y, 14X+xi]
  logits[b, Y, X, p] = sum_c patches * w[c] + ab
  l2 = logits * mask[b, Y, X] * wkk[p] + bkk[p]
  attn = softmax_p(l2)
  out[b, c, Y, X] = sum_p patches[..., p, c] * attn[p]

Sharding: 8 cores = 4 batches x 2 halves of the H(=Y) axis; per-core shard is
8 patch-rows x 16 X x 196 px x 384 c, shipped as fp16 (halves HBM traffic;
rel-err budget is 2e-2, fp16 end-to-end lands ~1e-3).

Per-core kernel, processed in 4 groups of 2 patch-rows (rX = 32 patches):
  - DMA 3 c-chunk tiles [128, 2, 16, 196] fp16
  - PE scoring: 96 matmuls into one PSUM tile lg[32, 196]; the one-hot
    stationary [128, 32] for column rX is a shifted window of a padded
    [128, 255] tensor (col 127 = w chunk), so LDWEIGHTS is 32 cols and the
    stationary library stays tiny.
  - batched softmax over p on [32, 196]: DVE affine (x2 TT), reduce-max,
    ACT exp (bias=-max, fp16 out, accum sum), DVE reciprocal + scale -> fp16
  - PE broadcast: for each patch, matmul(one-hot-row [32,128], attn[32,196])
    replicates that patch's attn over all 128 partitions; outputs land in
    [128, 4, 256]-padded PSUM tiles (each [128,196] slice bank-aligned)
  - ACT evacuates PSUM->SBUF fp16 in quarter-row [128, 4, 196] ops
  - DVE pass B per (chunk, row): one big multiply [128, 3136] (fp16 2x mode)
    + one segmented tensor_reduce(axis=X) [128, 16, 196] -> [128, 16] fp32
    written straight into the output accumulator tile.
"""

import sys

for _p in ("/opt/trn_rl_repo", "/root/.axon_site/_ro/trn_rl_repo"):
    if _p not in sys.path:
        sys.path.append(_p)

import numpy as np

import concourse.bacc as bacc
import concourse.mybir as mybir
import concourse.tile as tile
from concourse.bass_utils import run_bass_kernel_spmd

K = 14          # patch size
C = 384         # channels
CCH = 128       # channel chunk (partitions)
NCH = C // CCH  # 3 chunks
NX = 16         # patches across W
P = K * K       # 196 pixels per patch
NROW = 8        # patch rows per core
NCORES = 8
GR = 2          # rows per group
NG = NROW // GR
GP = GR * NX    # patches per group (32)

FP32 = mybir.dt.float32
FP16 = mybir.dt.float16


def build_nc():
    nc = bacc.Bacc("TRN2", target_bir_lowering=False, debug=False,
                   num_devices=NCORES)

    hr = nc.dram_tensor("hr", [C, NROW, NX, P], FP16, kind="ExternalInput")
    # padded one-hot scorer weights: woh[c, k, 127] = w16[k*128+c], else 0
    woh = nc.dram_tensor("woh", [CCH, NCH, 255], FP16, kind="ExternalInput")
    # one-hot row selectors for the broadcast: bc[q, t, m] = (q == t)
    bc = nc.dram_tensor("bc", [GP, GP, CCH], FP16, kind="ExternalInput")
    # [GP, NG, P]: group index on the free axis so every DVE op reads
    # partitions 0..31 (DVE lanes are partition-hardwired)
    mwB = nc.dram_tensor("mwB", [GP, NG, P], FP32, kind="ExternalInput")
    bkkB = nc.dram_tensor("bkkB", [GP, NG, P], FP32, kind="ExternalInput")
    out = nc.dram_tensor("out", [C, NROW, NX], FP32, kind="ExternalOutput")

    with tile.TileContext(nc) as tc:
        _emit(tc, nc, hr, woh, bc, mwB, bkkB, out)
    nc.finalize()
    return nc


def _emit(tc, nc, hr, woh, bc, mwB, bkkB, out):
    import contextlib
    ctx = contextlib.ExitStack()
    with ctx:
        singles = ctx.enter_context(tc.tile_pool(name="singles", bufs=1))
        data_pool = ctx.enter_context(tc.tile_pool(name="data", bufs=9))
        small = ctx.enter_context(tc.tile_pool(name="small", bufs=8))
        attnb_pool = ctx.enter_context(tc.tile_pool(name="attnb", bufs=3))
        prod_pool = ctx.enter_context(tc.tile_pool(name="prod", bufs=3))
        prod2_pool = ctx.enter_context(tc.tile_pool(name="prod2", bufs=3))
        scratch_pool = ctx.enter_context(tc.tile_pool(name="scratch", bufs=1))
        psum_lg = ctx.enter_context(
            tc.tile_pool(name="psum_lg", bufs=3, space="PSUM"))
        psum_bc = ctx.enter_context(
            tc.tile_pool(name="psum_bc", bufs=2, space="PSUM"))

        # ---- constants (loaded once) ----
        woh_sb = singles.tile([CCH, NCH, 255], FP16)
        nc.sync.dma_start(out=woh_sb, in_=woh[:, :, :])
        bc_sb = singles.tile([GP, GP, CCH], FP16)
        nc.sync.dma_start(out=bc_sb, in_=bc[:, :, :])
        mwB_sb = singles.tile([GP, NG, P], FP32)
        nc.sync.dma_start(out=mwB_sb, in_=mwB[:, :, :])
        bkkB_sb = singles.tile([GP, NG, P], FP32)
        nc.sync.dma_start(out=bkkB_sb, in_=bkkB[:, :, :])

        osb = singles.tile([CCH, NCH, NROW, NX], FP32)
        act_scr = scratch_pool.tile([CCH, P // 2], FP32, tag="act_scr")

        dk_all = {}
        attn_all = {}

        def front_half(g):
            """DMA + scoring for group g (keeps PE fed ahead of the DVE)."""
            r0 = GR * g
            dk = []
            for k in range(NCH):
                t = data_pool.tile([CCH, GR, NX, P], FP16, tag="data")
                nc.sync.dma_start(
                    out=t, in_=hr[k * CCH:(k + 1) * CCH, r0:r0 + GR, :, :])
                dk.append(t)
            dk_all[g] = dk

            # k-outer: the first chunk's matmuls can start as soon as that
            # chunk's DMA lands (per-element start/stop accumulation flags)
            lg = psum_lg.tile([GP, P], FP32, tag="lg")
            for k in range(NCH):
                for ri in range(GR):
                    for X in range(NX):
                        col = NX * ri + X
                        nc.tensor.matmul(
                            lg[:, :],
                            woh_sb[:, k, 127 - col:127 - col + GP],
                            dk[k][:, ri, X, :],
                            start=(k == 0 and ri == 0 and X == 0),
                            stop=(k == NCH - 1 and ri == GR - 1
                                  and X == NX - 1),
                        )
            return lg

        def softmax_group(g, lg):
            l2 = small.tile([GP, P], FP32, tag="l2")
            nc.vector.tensor_mul(l2, lg[:, :], mwB_sb[:, g, :])
            nc.vector.tensor_add(l2, l2, bkkB_sb[:, g, :])
            negmax = small.tile([GP, 1], FP32, tag="negmax")
            nc.vector.tensor_reduce(negmax, l2, axis=mybir.AxisListType.X,
                                    op=mybir.AluOpType.max, negate=True)
            ex16 = small.tile([GP, P], FP16, tag="ex16")
            esum = small.tile([GP, 1], FP32, tag="esum")
            nc.scalar.activation(ex16, l2, mybir.ActivationFunctionType.Exp,
                                 bias=negmax[:, 0:1], scale=1.0,
                                 accum_out=esum[:, 0:1])
            rcp = small.tile([GP, 1], FP32, tag="rcp")
            nc.vector.reciprocal(rcp, esum)
            attn16 = small.tile([GP, P], FP16, tag="attn16")
            nc.vector.tensor_scalar_mul(attn16, ex16, rcp[:, 0:1])
            return attn16

        def back_half(g, attn16):
            """broadcast + evac + multiply/fold/reduce for group g."""
            dk = dk_all[g]
            for ri in range(GR):
                r = GR * g + ri
                aB = attnb_pool.tile([CCH, NX, P], FP16, tag="aB")
                for q in range(4):          # quarter-rows of 4 patches
                    abp = psum_bc.tile([CCH, 4, 256], FP32, tag="abp")
                    for j in range(4):
                        t_loc = NX * ri + 4 * q + j
                        nc.tensor.matmul(
                            abp[:, j, 0:P],
                            bc_sb[:, t_loc, :],
                            attn16[:, :],
                            start=True, stop=True,
                        )
                    nc.scalar.activation(
                        aB[:, 4 * q:4 * q + 4, :], abp[:, :, 0:P],
                        mybir.ActivationFunctionType.Copy)

                for k in range(NCH):
                    prod = prod_pool.tile([CCH, NX, P], FP16, tag="prod")
                    nc.vector.tensor_mul(prod, dk[k][:, ri, :, :], aB)
                    prod2 = prod2_pool.tile([CCH, NX, P // 2], FP16,
                                            tag="prod2")
                    nc.vector.tensor_add(prod2, prod[:, :, 0:P // 2],
                                         prod[:, :, P // 2:P])
                    nc.vector.tensor_reduce(
                        osb[:, k, r, :], prod2, axis=mybir.AxisListType.X,
                        op=mybir.AluOpType.add)

        # depth-2 software pipeline: PE scores two groups ahead so the DVE
        # never waits on scoring, and scoring never queues behind broadcasts
        lgs = {}
        for g in range(NG + 2):
            if g < NG:
                lgs[g] = front_half(g)
            if g >= 2:
                attn16 = softmax_group(g - 2, lgs.pop(g - 2))
                back_half(g - 2, attn16)

        for k in range(NCH):
            nc.sync.dma_start(out=out[k * CCH:(k + 1) * CCH, :, :],
                              in_=osb[:, k, :, :])


_NC_CACHE = {}


def _get_nc():
    if "nc" not in _NC_CACHE:
        _NC_CACHE["nc"] = build_nc()
    return _NC_CACHE["nc"]


def make_in_maps(hr_feats, guidance, attn_w, attn_b, w_kk, b_kk, dropout_mask):
    b = hr_feats.shape[0]
    w16 = np.asarray(attn_w, np.float32)[0].astype(np.float16)    # [384]
    ab = np.float32(np.asarray(attn_b, np.float32)[0])
    wkk_flat = np.asarray(w_kk, np.float32).reshape(-1)           # [196]
    bkk_flat = np.asarray(b_kk, np.float32).reshape(-1)
    mask = np.asarray(dropout_mask).astype(np.float32)[..., 0]    # [b, H, W]

    woh = np.zeros((CCH, NCH, 255), np.float16)
    woh[:, :, 127] = w16.reshape(NCH, CCH).T
    bc = np.zeros((GP, GP, CCH), np.float16)
    bc[np.arange(GP), np.arange(GP), :] = np.float16(1.0)

    in_maps = []
    for core in range(NCORES):
        bi, half = divmod(core, 2)
        bi = bi % b
        sl = np.asarray(hr_feats[bi, :, 112 * half:112 * half + K * NROW, :],
                        np.float32)
        hrg = sl.reshape(C, NROW, K, NX, K).transpose(0, 1, 3, 2, 4)
        hrg = np.ascontiguousarray(
            hrg.reshape(C, NROW, NX, P), np.float16)
        mask_flat = np.ascontiguousarray(
            mask[bi, NROW * half:NROW * half + NROW, :]).reshape(-1)  # [128]
        mwB = (mask_flat[:, None] * wkk_flat[None, :]).astype(np.float32)
        bkkB = (ab * mwB + bkk_flat[None, :]).astype(np.float32)
        # [128, 196] -> [GP, NG, P] with group index on the free axis
        mwB = np.ascontiguousarray(
            mwB.reshape(NG, GP, P).transpose(1, 0, 2))
        bkkB = np.ascontiguousarray(
            bkkB.reshape(NG, GP, P).transpose(1, 0, 2))
        in_maps.append({
            "hr": hrg, "woh": woh, "bc": bc, "mwB": mwB, "bkkB": bkkB,
        })
    return in_maps


def kernel(hr_feats, guidance, attn_w, attn_b, w_kk, b_kk, dropout_mask,
           trace=False):
    hr_feats = np.asarray(hr_feats, np.float32)
    b = hr_feats.shape[0]
    H = hr_feats.shape[2] // K
    nc = _get_nc()
    in_maps = make_in_maps(hr_feats, guidance, attn_w, attn_b, w_kk, b_kk,
                           dropout_mask)
    res = run_bass_kernel_spmd(nc, in_maps, core_ids=list(range(NCORES)),
                               trace=trace)
    full = np.empty((b, C, H, NX), np.float32)
    for core in range(NCORES):
        bi, half = divmod(core, 2)
        full[bi, :, NROW * half:NROW * half + NROW, :] = \
            res.results[core]["out"]
    if trace:
        return full, res
    return full
